# revision 11
# baseline (speedup 1.0000x reference)
"""MLA QKV projection kernel v3 for Trainium2 (8 NeuronCores, Bass/Tile).

Strategy vs v2 baseline (643.3 us):
- Partial fp8: the first 4 k-tiles (of 12) of the B-q GEMM and ALL 4
  k-tiles of the B-kv GEMM run as fp8e4 DoubleRow matmuls (2 k-tiles
  contracted per call at bf16 per-call cost = 2x throughput). Saves
  ~99 us of PE time. Offline-calibrated rel_err 1.53e-2 < 2e-2 gate.
  Scales: at evicted as at*16 (both bf16 and e4m3), B weights hosted
  as w*1024; psum = (at@w)*2^14, descaled by folding 2^-14 into the
  rsqrt scale (Sqrt activation scale=2^-28).
- Outputs bf16 in partition-major layout [*, 128, MT, D]: contiguous
  2-3KB DMA descriptors per partition (v2's token-major rearrange made
  512B descriptors that left a 13 us descriptor-bound drain tail).
  Host upcasts/transposes for free.
- Head: first hst/wa tiles subtiled so the first matmul starts at
  ~10 us instead of 20.8; PE warmed up with dummy matmuls during the
  initial DMA wait (pstate ramp).
- RMSNorm sum-of-squares taken from the A psum (not the evicted bf16
  copy), squares via DVE, partition-reduction via per-(cb,m)
  ones-matmuls into token-partition layout [128, MT] as in v2.
"""

import sys
import types

import numpy as np

# ---- constants (hardcoded problem shape) ----
H = 32
D_NOPE = 128
D_ROPE = 64
D_Q = 192
D_V = 128
R_KV = 512
RQ = 1536
DMODEL = 4096
EPS = 1e-6
B, S = 2, 4096
NTOK = B * S            # 8192
NCORES = 8
TPC = NTOK // NCORES    # 1024 tokens per core
MT = TPC // 128         # 8 m-tiles
KT = DMODEL // 128      # 32 k-tiles for the A GEMM

A_COLS = RQ + R_KV + D_ROPE    # 2112
ACB = 17                       # a col-blocks of 128 (2176 padded; cb16 = k_pe+pad)
QKT = RQ // 128                # 12 contraction tiles for B-q
KVKT = R_KV // 128             # 4 for B-kv
QCH = 24                       # B-q chunks of 256 out cols (6144)
KVCH = 32                      # B-kv chunks of 256 out cols = 1 head (8192)

N8_BQ = 2                      # fp8 DoubleRow k-pairs in B-q (k-tiles 0..3)
N8_BKV = 2                     # fp8 DoubleRow k-pairs in B-kv (all 4 k-tiles)
QKB = QKT - 2 * N8_BQ          # bf16 k-tiles in B-q (8)
SA = 16.0                      # at fp8/bf16 eviction scale
SWB = 1024.0                   # B-weight host scale
SSQRT = 2.0 ** -28             # folds 1/(SA*SWB) into the rsqrt


def _ensure_env():
    for p in ("/opt/trn_rl_repo", "/root/.axon_site"):
        if p not in sys.path:
            sys.path.insert(0, p)
    if "antenv.axon_hooks" not in sys.modules:
        try:
            import antenv  # noqa: F401
            import antenv.axon_hooks  # noqa: F401
        except ImportError:
            mod = types.ModuleType("antenv.axon_hooks")
            mod._hook = None
            mod.set_axon_ntff_profile_hook = lambda h: setattr(mod, "_hook", h)
            mod.get_axon_ntff_profile_hook = lambda: mod._hook
            sys.modules["antenv.axon_hooks"] = mod
            try:
                import antenv
                antenv.axon_hooks = mod
            except ImportError:
                pass


def _perm64():
    # inverse view of x.reshape(32,2).swapaxes -> y[k] = x[2*(k%32) + k//32]
    return np.array([2 * (k % 32) + k // 32 for k in range(64)], dtype=np.int64)


_CACHE = {}


def _build():
    if "nc" in _CACHE:
        return _CACHE["nc"]
    _ensure_env()
    from concourse import bacc
    import concourse.mybir as mybir
    import concourse.tile as tile

    F32 = mybir.dt.float32
    BF16 = mybir.dt.bfloat16
    F8 = mybir.dt.float8e4
    AF = mybir.ActivationFunctionType
    ALU = mybir.AluOpType
    DR = mybir.MatmulPerfMode.DoubleRow

    nc = bacc.Bacc("TRN2", target_bir_lowering=False, debug=False)
    hsT_d = nc.dram_tensor("hsT", [8, 128, 4, TPC], BF16, kind="ExternalInput")
    waT_d = nc.dram_tensor("waT", [ACB, 128, KT, 128], BF16, kind="ExternalInput")
    qbT_d = nc.dram_tensor("qbT", [QCH, 128, QKB, 256], BF16, kind="ExternalInput")
    qb8_d = nc.dram_tensor("qb8", [QCH, 128, N8_BQ, 2, 256], F8, kind="ExternalInput")
    kvb8_d = nc.dram_tensor("kvb8", [KVCH, 128, N8_BKV, 2, 256], F8,
                            kind="ExternalInput")
    oq_d = nc.dram_tensor("oq", [H, 128, MT, D_Q], BF16, kind="ExternalOutput")
    okn_d = nc.dram_tensor("okn", [H, 128, MT, D_NOPE], BF16, kind="ExternalOutput")
    ov_d = nc.dram_tensor("ov", [H, 128, MT, D_V], BF16, kind="ExternalOutput")
    ope_d = nc.dram_tensor("ope", [D_ROPE, TPC], BF16, kind="ExternalOutput")

    with tile.TileContext(nc) as tc:
        with tc.tile_pool(name="persist", bufs=1) as persist, \
             tc.tile_pool(name="wq", bufs=3) as wqp, \
             tc.tile_pool(name="wkv", bufs=3) as wkvp:
            at_sb = persist.tile([128, QKB, TPC], BF16)        # bf16 at (q k4-11)
            at8q = persist.tile([128, N8_BQ, 2, TPC], F8)      # fp8 at (q k0-3)
            at8kv = persist.tile([128, N8_BKV, 2, TPC], F8)    # fp8 at (kv k0-3)
            s_q = persist.tile([128, MT], F32)
            s_kv = persist.tile([128, MT], F32)
            ones = persist.tile([128, 1], BF16)
            warm = persist.tile([128, 512], BF16)
            nc.vector.memset(ones, 1.0)
            nc.vector.memset(warm, 0.001)

            # ---------------- phase A: col-major A GEMM (all bf16) ----------------
            wq_pre = {}
            wq8_pre = {}
            wkv8_pre = {}
            with tc.tile_pool(name="hst", bufs=1) as hstp, \
                 tc.tile_pool(name="wa", bufs=3) as wap, \
                 tc.tile_pool(name="sq", bufs=3) as sqp, \
                 tc.tile_pool(name="pe", bufs=1) as pep, \
                 tc.tile_pool(name="psA", bufs=3, space="PSUM") as psA, \
                 tc.tile_pool(name="sps", bufs=1, space="PSUM") as spsp:
                # PE warmup: dummy chained matmuls ramp the tensor engine's
                # pstate while the first input DMAs stream in
                ps_w = psA.tile([128, TPC], F32, tag="psA", name="psA_warm")
                for i in range(10):
                    nc.tensor.matmul(ps_w[:, 0:512], warm[:, 0:128], warm,
                                     start=(i == 0), stop=(i == 9),
                                     skip_group_check=True)

                hst = [hstp.tile([128, 4, TPC], BF16, name=f"hst{i}") for i in range(8)]
                # first-needed data split small and spread across queues so the
                # first matmul starts ~10us: cb0 k0-1 needs hst0a + wa0a only
                nc.sync.dma_start(out=hst[0][:, 0:2, :], in_=hsT_d[0][:, 0:2, :])
                nc.gpsimd.dma_start(out=hst[0][:, 2:4, :], in_=hsT_d[0][:, 2:4, :])
                for i in range(1, 8):
                    nc.scalar.dma_start(out=hst[i], in_=hsT_d[i])
                sps_q = spsp.tile([128, MT], F32)
                sps_kv = spsp.tile([128, MT], F32)
                sq_tiles = {}

                def stats_mms(pcb):
                    sq_t = sq_tiles.pop(pcb)
                    tgt, g0, g1 = (sps_q, 0, QKT - 1) if pcb < QKT else (sps_kv, QKT, 15)
                    for m in range(MT):
                        # start only on the bank's first write: start_tensor_calc
                        # clears has_written for the whole 2KB bank
                        nc.tensor.matmul(tgt[:, m:m + 1], sq_t[:, m * 128:(m + 1) * 128],
                                         ones, start=(pcb == g0 and m == 0),
                                         stop=(pcb == g1),
                                         skip_group_check=True)

                def a_mms(ps, wa_t, k0, k1):
                    # j outer: consecutive matmuls then use different stationary
                    # tiles, so LDWEIGHTS alternates weight buffers and hides
                    for j in range(2):
                        for k in range(k0, k1):
                            nc.tensor.matmul(ps[:, j * 512:(j + 1) * 512],
                                             wa_t[:, k, :],
                                             hst[k // 4][:, k % 4, j * 512:(j + 1) * 512],
                                             start=(k == 0), stop=(k == KT - 1),
                                             skip_group_check=True)

                def a_evict(cb, ps):
                    # at evicted as at*16: bf16 for B-q's bf16 k-tiles, fp8
                    # elsewhere; the 16*1024 product scale is folded into the
                    # rsqrt scale at B eviction
                    if cb < 2 * N8_BQ:
                        tgt = at8q[:, cb // 2, cb % 2, :]
                    elif cb < QKT:
                        tgt = at_sb[:, cb - 2 * N8_BQ, :]
                    else:
                        tgt = at8kv[:, (cb - QKT) // 2, (cb - QKT) % 2, :]
                    nc.scalar.activation(tgt, ps, AF.Copy, scale=SA)
                    sq_t = sqp.tile([128, TPC], BF16, tag="sq")
                    nc.scalar.activation(sq_t, ps, AF.Square)
                    sq_tiles[cb] = sq_t

                # cb0-2 interleaved in k-thirds: cuts the early hst consumption
                # rate so the initial hst DMA stream keeps up; cb0's first seg
                # further split so compute starts on hst0a+wa0a (0.5MB)
                was, pss = [], []
                for cb in range(3):
                    t = wap.tile([128, KT, 128], BF16, tag="wa", name=f"wa_s{cb}")
                    if cb == 0:
                        nc.sync.dma_start(out=t[:, 0:8, :], in_=waT_d[0][:, 0:8, :])
                        nc.gpsimd.dma_start(out=t[:, 8:, :], in_=waT_d[0][:, 8:, :])
                    else:
                        nc.sync.dma_start(out=t, in_=waT_d[cb])
                    was.append(t)
                    pss.append(psA.tile([128, TPC], F32, tag="psA", name=f"psA_s{cb}"))
                for seg, (k0, k1) in enumerate(((0, 2), (2, 8), (8, 11),
                                               (11, 22), (22, KT))):
                    for cb in range(3):
                        if seg < 2 and cb > 0:
                            continue        # cb1/cb2 start at seg2 (k0-10 whole)
                        if seg == 2 and cb > 0:
                            a_mms(pss[cb], was[cb], 0, 11)
                            continue
                        a_mms(pss[cb], was[cb], k0, k1)
                        if seg == 4:
                            a_evict(cb, pss[cb])
                stats_mms(0)
                stats_mms(1)

                for cb in range(3, ACB):
                    wa_t = wap.tile([128, KT, 128], BF16, tag="wa")
                    nc.sync.dma_start(out=wa_t, in_=waT_d[cb])
                    if cb == 14:
                        # prefetch the first B-phase weight tiles so phase B
                        # matmuls start without a DMA stall
                        for c in range(3):
                            t8 = wqp.tile([128, N8_BQ, 2, 256], F8, tag="wq8")
                            nc.sync.dma_start(out=t8, in_=qb8_d[c])
                            wq8_pre[c] = t8
                            t = wqp.tile([128, QKB, 256], BF16, tag="wq")
                            nc.sync.dma_start(out=t, in_=qbT_d[c])
                            wq_pre[c] = t
                        t8 = wkvp.tile([128, N8_BKV, 2, 256], F8, tag="wkv8")
                        nc.sync.dma_start(out=t8, in_=kvb8_d[0])
                        wkv8_pre[0] = t8
                    ps = psA.tile([128, TPC], F32, tag="psA")
                    a_mms(ps, wa_t, 0, KT)
                    stats_mms(cb - 1)       # slack: evict+square done ~13us ago
                    if cb < 16:
                        a_evict(cb, ps)
                    else:
                        pe_sb = pep.tile([128, TPC], BF16)
                        nc.scalar.activation(pe_sb[0:D_ROPE, :], ps[0:D_ROPE, :], AF.Copy)
                        nc.gpsimd.dma_start(out=ope_d[:, :], in_=pe_sb[0:D_ROPE, :])

                # per-token scales in [128 tok, m] layout; Sqrt scale 2^-28
                # bakes the 1/(SA*SWB) descale into the per-token rsqrt
                for sps, s_tok, dim in ((sps_q, s_q, float(RQ)), (sps_kv, s_kv, float(R_KV))):
                    nc.vector.tensor_scalar(out=s_tok, in0=sps, scalar1=1.0 / dim,
                                            scalar2=EPS, op0=ALU.mult, op1=ALU.add)
                    nc.vector.reciprocal(s_tok, s_tok)
                    nc.scalar.activation(s_tok, s_tok, AF.Sqrt, scale=SSQRT)

            # ---------------- phase B: token-major B GEMMs, q/kv interleaved ----------------
            with tc.tile_pool(name="stage", bufs=2) as stp, \
                 tc.tile_pool(name="ev", bufs=3) as evp, \
                 tc.tile_pool(name="psB", bufs=2, space="PSUM") as psB:
                stage = None
                for i in range(8):
                    for c in range(3 * i, 3 * i + 3):       # 3 q chunks (256 cols each)
                        if c in wq_pre:
                            wq_t = wq_pre.pop(c)
                            wq8_t = wq8_pre.pop(c)
                        else:
                            wq8_t = wqp.tile([128, N8_BQ, 2, 256], F8, tag="wq8")
                            nc.sync.dma_start(out=wq8_t, in_=qb8_d[c])
                            wq_t = wqp.tile([128, QKB, 256], BF16, tag="wq")
                            nc.sync.dma_start(out=wq_t, in_=qbT_d[c])
                        ps = psB.tile([128, MT, 256], F32, tag="psB")
                        for p in range(N8_BQ):
                            for m in range(MT):
                                # m-slices are 1KB: two share a 2KB psum bank, so
                                # only the even m's first matmul may set start
                                nc.tensor.matmul(ps[:, m, :],
                                                 at8q[:, p, :, m * 128:(m + 1) * 128],
                                                 wq8_t[:, p],
                                                 start=(p == 0 and m % 2 == 0),
                                                 stop=False, perf_mode=DR,
                                                 skip_group_check=True)
                        for k in range(QKB):
                            for m in range(MT):
                                nc.tensor.matmul(ps[:, m, :],
                                                 at_sb[:, k, m * 128:(m + 1) * 128],
                                                 wq_t[:, k, :],
                                                 start=False, stop=(k == QKB - 1),
                                                 skip_group_check=True)
                        if c % 3 == 0:
                            stage = stp.tile([128, MT, 768], BF16, tag="stage")
                        o0 = (c % 3) * 256
                        # scaled eviction: scalar does m0-2, vector m3-7; the
                        # 3/5 split keeps vector (the psum-release critical
                        # path during kv stretches) under the chunk matmul time
                        for m in range(3):
                            nc.scalar.activation(stage[:, m, o0:o0 + 256], ps[:, m, :],
                                                 AF.Copy, scale=s_q[:, m:m + 1])
                        nc.vector.tensor_tensor(
                            out=stage[:, 3:8, o0:o0 + 256], in0=ps[:, 3:8, :],
                            in1=s_q[:, 3:8, None].broadcast_to([128, 5, 256]),
                            op=ALU.mult)
                        # heads complete progressively: c%3==0 -> h+0, ==1 -> h+1,
                        # ==2 -> h+2 and h+3 (keeps the output queue smooth)
                        h0 = (c // 3) * 4
                        for hh in ([0], [1], [2, 3])[c % 3]:
                            nc.gpsimd.dma_start(
                                out=oq_d[h0 + hh],
                                in_=stage[:, :, hh * D_Q:(hh + 1) * D_Q])
                    for c in range(4 * i, 4 * i + 4):       # 4 kv chunks (1 head each)
                        if c in wkv8_pre:
                            wkv8_t = wkv8_pre.pop(c)
                        else:
                            wkv8_t = wkvp.tile([128, N8_BKV, 2, 256], F8, tag="wkv8")
                            nc.sync.dma_start(out=wkv8_t, in_=kvb8_d[c])
                        ps = psB.tile([128, MT, 256], F32, tag="psB")
                        for p in range(N8_BKV):
                            for m in range(MT):
                                nc.tensor.matmul(ps[:, m, :],
                                                 at8kv[:, p, :, m * 128:(m + 1) * 128],
                                                 wkv8_t[:, p],
                                                 start=(p == 0 and m % 2 == 0),
                                                 stop=(p == N8_BKV - 1), perf_mode=DR,
                                                 skip_group_check=True)
                        ev = evp.tile([128, MT, 256], BF16, tag="ev")
                        msplit = 4 if c == KVCH - 1 else 3
                        for m in range(msplit):
                            nc.scalar.activation(ev[:, m, :], ps[:, m, :],
                                                 AF.Copy, scale=s_kv[:, m:m + 1])
                        nc.vector.tensor_tensor(
                            out=ev[:, msplit:8, :], in0=ps[:, msplit:8, :],
                            in1=s_kv[:, msplit:8, None].broadcast_to([128, 8 - msplit, 256]),
                            op=ALU.mult)
                        # spread the tail chunks' output DMAs across queues so
                        # the drain after the last matmul stays short
                        okn_eng = nc.scalar if c >= KVCH - 3 else nc.gpsimd
                        ov_eng = nc.sync if c >= KVCH - 2 else nc.scalar
                        okn_eng.dma_start(out=okn_d[c], in_=ev[:, :, 0:D_NOPE])
                        ov_eng.dma_start(out=ov_d[c], in_=ev[:, :, D_NOPE:256])

    nc.compile()
    _CACHE["nc"] = nc
    return nc


def _prep_inputs(hidden_states, q_a_w, kv_a_w, q_b_w, kv_b_w, q_a_ln_w, kv_a_ln_w):
    import ml_dtypes
    f32 = np.float32
    bf16 = ml_dtypes.bfloat16
    e4m3 = ml_dtypes.float8_e4m3
    perm = _perm64()

    q_a_w = np.asarray(q_a_w, dtype=f32)
    kv_a_w = np.asarray(kv_a_w, dtype=f32)
    wa = np.zeros((ACB * 128, DMODEL), dtype=f32)
    wa[:RQ] = q_a_w
    wa[RQ:RQ + R_KV] = kv_a_w[:R_KV]
    wa[RQ + R_KV:A_COLS] = kv_a_w[R_KV:][perm]             # de-interleave k_pe rows
    waT = np.ascontiguousarray(
        wa.reshape(ACB, 128, KT, 128).transpose(0, 3, 2, 1)).astype(bf16)

    qb = np.asarray(q_b_w, dtype=f32) * np.asarray(q_a_ln_w, dtype=f32)[None, :]
    qb = qb.reshape(H, D_Q, RQ).copy()
    qb[:, D_NOPE:, :] = qb[:, D_NOPE + perm, :]            # de-interleave q_pe rows
    qbt = (qb.reshape(QCH, 256, QKT, 128) * SWB).transpose(0, 3, 2, 1)
    qbT = np.ascontiguousarray(qbt[:, :, 2 * N8_BQ:, :]).astype(bf16)
    qb8 = np.clip(qbt[:, :, :2 * N8_BQ, :], -240, 240).astype(e4m3).reshape(
        QCH, 128, N8_BQ, 2, 256)
    qb8 = np.ascontiguousarray(qb8)

    kvb = np.asarray(kv_b_w, dtype=f32) * np.asarray(kv_a_ln_w, dtype=f32)[None, :]
    kvbt = (kvb.reshape(KVCH, 256, KVKT, 128) * SWB).transpose(0, 3, 2, 1)
    kvb8 = np.clip(kvbt, -240, 240).astype(e4m3).reshape(KVCH, 128, N8_BKV, 2, 256)
    kvb8 = np.ascontiguousarray(kvb8)

    hs = np.asarray(hidden_states, dtype=f32).reshape(NTOK, DMODEL)
    in_maps = []
    for c in range(NCORES):
        # hsT_d[i, p, kk, t] = hs[tok0+t, (i*4+kk)*128 + p]
        hsc = np.ascontiguousarray(
            hs[c * TPC:(c + 1) * TPC].reshape(TPC, KT, 128).transpose(2, 1, 0)
            .reshape(128, 8, 4, TPC).transpose(1, 0, 2, 3)).astype(bf16)
        in_maps.append({"hsT": hsc, "waT": waT, "qbT": qbT, "qb8": qb8,
                        "kvb8": kvb8})
    return in_maps


def kernel(hidden_states, q_a_w, q_b_w, kv_a_w, kv_b_w, q_a_ln_w, kv_a_ln_w,
           _trace=False):
    _ensure_env()
    from concourse.bass_utils import run_bass_kernel_spmd

    nc = _build()
    in_maps = _prep_inputs(hidden_states, q_a_w, kv_a_w, q_b_w, kv_b_w,
                           q_a_ln_w, kv_a_ln_w)
    res = run_bass_kernel_spmd(nc, in_maps, list(range(NCORES)), trace=_trace)

    out = np.empty((B, 3 * H, S, D_Q), dtype=np.float32)
    for c in range(NCORES):
        r = res.results[c]
        b = c // (S // TPC)
        sl = slice((c % (S // TPC)) * TPC, ((c % (S // TPC)) + 1) * TPC)
        # device layout [*, 128, MT, D]: token = m*128 + p
        out[b, :H, sl, :] = r["oq"].transpose(0, 2, 1, 3).reshape(
            H, TPC, D_Q).astype(np.float32)
        out[b, H:2 * H, sl, :D_NOPE] = r["okn"].transpose(0, 2, 1, 3).reshape(
            H, TPC, D_NOPE).astype(np.float32)
        out[b, H:2 * H, sl, D_NOPE:] = r["ope"].T.astype(np.float32)[None, :, :]
        out[b, 2 * H:, sl, :D_V] = r["ov"].transpose(0, 2, 1, 3).reshape(
            H, TPC, D_V).astype(np.float32)
    out[:, 2 * H:, :, D_V:] = 0.0
    if _trace:
        kernel.last_exec_time_ns = res.exec_time_ns
        kernel.last_results = res
    return out


# revision 12
# speedup vs baseline: 1.0217x; 1.0217x over previous
"""MLA QKV projection kernel v3 for Trainium2 (8 NeuronCores, Bass/Tile).

Strategy vs v2 baseline (643.3 us):
- Partial fp8: the first 4 k-tiles (of 12) of the B-q GEMM and ALL 4
  k-tiles of the B-kv GEMM run as fp8e4 DoubleRow matmuls (2 k-tiles
  contracted per call at bf16 per-call cost = 2x throughput). Saves
  ~99 us of PE time. Offline-calibrated rel_err 1.53e-2 < 2e-2 gate.
  Scales: at evicted as at*16 (both bf16 and e4m3), B weights hosted
  as w*1024; psum = (at@w)*2^14, descaled by folding 2^-14 into the
  rsqrt scale (Sqrt activation scale=2^-28).
- Outputs bf16 in partition-major layout [*, 128, MT, D]: contiguous
  2-3KB DMA descriptors per partition (v2's token-major rearrange made
  512B descriptors that left a 13 us descriptor-bound drain tail).
  Host upcasts/transposes for free.
- Head: first hst/wa tiles subtiled so the first matmul starts at
  ~10 us instead of 20.8; PE warmed up with dummy matmuls during the
  initial DMA wait (pstate ramp).
- RMSNorm sum-of-squares taken from the A psum (not the evicted bf16
  copy), squares via DVE, partition-reduction via per-(cb,m)
  ones-matmuls into token-partition layout [128, MT] as in v2.
"""

import sys
import types

import numpy as np

# ---- constants (hardcoded problem shape) ----
H = 32
D_NOPE = 128
D_ROPE = 64
D_Q = 192
D_V = 128
R_KV = 512
RQ = 1536
DMODEL = 4096
EPS = 1e-6
B, S = 2, 4096
NTOK = B * S            # 8192
NCORES = 8
TPC = NTOK // NCORES    # 1024 tokens per core
MT = TPC // 128         # 8 m-tiles
KT = DMODEL // 128      # 32 k-tiles for the A GEMM

A_COLS = RQ + R_KV + D_ROPE    # 2112
ACB = 17                       # a col-blocks of 128 (2176 padded; cb16 = k_pe+pad)
QKT = RQ // 128                # 12 contraction tiles for B-q
KVKT = R_KV // 128             # 4 for B-kv
QCH = 24                       # B-q chunks of 256 out cols (6144)
KVCH = 32                      # B-kv chunks of 256 out cols = 1 head (8192)

N8_BQ = 2                      # fp8 DoubleRow k-pairs in B-q (k-tiles 0..3)
N8_BKV = 2                     # fp8 DoubleRow k-pairs in B-kv (all 4 k-tiles)
QKB = QKT - 2 * N8_BQ          # bf16 k-tiles in B-q (8)
SA = 16.0                      # at fp8/bf16 eviction scale
SWB = 1024.0                   # B-weight host scale
SSQRT = 2.0 ** -28             # folds 1/(SA*SWB) into the rsqrt


def _ensure_env():
    for p in ("/opt/trn_rl_repo", "/root/.axon_site"):
        if p not in sys.path:
            sys.path.insert(0, p)
    if "antenv.axon_hooks" not in sys.modules:
        try:
            import antenv  # noqa: F401
            import antenv.axon_hooks  # noqa: F401
        except ImportError:
            mod = types.ModuleType("antenv.axon_hooks")
            mod._hook = None
            mod.set_axon_ntff_profile_hook = lambda h: setattr(mod, "_hook", h)
            mod.get_axon_ntff_profile_hook = lambda: mod._hook
            sys.modules["antenv.axon_hooks"] = mod
            try:
                import antenv
                antenv.axon_hooks = mod
            except ImportError:
                pass


def _perm64():
    # inverse view of x.reshape(32,2).swapaxes -> y[k] = x[2*(k%32) + k//32]
    return np.array([2 * (k % 32) + k // 32 for k in range(64)], dtype=np.int64)


_CACHE = {}


def _build():
    if "nc" in _CACHE:
        return _CACHE["nc"]
    _ensure_env()
    from concourse import bacc
    import concourse.mybir as mybir
    import concourse.tile as tile

    F32 = mybir.dt.float32
    BF16 = mybir.dt.bfloat16
    F8 = mybir.dt.float8e4
    AF = mybir.ActivationFunctionType
    ALU = mybir.AluOpType
    DR = mybir.MatmulPerfMode.DoubleRow

    nc = bacc.Bacc("TRN2", target_bir_lowering=False, debug=False)
    hsT_d = nc.dram_tensor("hsT", [8, 128, 4, TPC], BF16, kind="ExternalInput")
    waT_d = nc.dram_tensor("waT", [ACB, 128, KT, 128], BF16, kind="ExternalInput")
    qbT_d = nc.dram_tensor("qbT", [QCH, 128, QKB, 256], BF16, kind="ExternalInput")
    qb8_d = nc.dram_tensor("qb8", [QCH, 128, N8_BQ, 2, 256], F8, kind="ExternalInput")
    kvb8_d = nc.dram_tensor("kvb8", [KVCH, 128, N8_BKV, 2, 256], F8,
                            kind="ExternalInput")
    oq_d = nc.dram_tensor("oq", [H, 128, MT, D_Q], BF16, kind="ExternalOutput")
    okn_d = nc.dram_tensor("okn", [H, 128, MT, D_NOPE], BF16, kind="ExternalOutput")
    ov_d = nc.dram_tensor("ov", [H, 128, MT, D_V], BF16, kind="ExternalOutput")
    ope_d = nc.dram_tensor("ope", [D_ROPE, TPC], BF16, kind="ExternalOutput")

    with tile.TileContext(nc) as tc:
        with tc.tile_pool(name="persist", bufs=1) as persist, \
             tc.tile_pool(name="wq", bufs=3) as wqp, \
             tc.tile_pool(name="wkv", bufs=3) as wkvp:
            at_sb = persist.tile([128, QKB, TPC], BF16)        # bf16 at (q k4-11)
            at8q = persist.tile([128, N8_BQ, 2, TPC], F8)      # fp8 at (q k0-3)
            at8kv = persist.tile([128, N8_BKV, 2, TPC], F8)    # fp8 at (kv k0-3)
            s_q = persist.tile([128, MT], F32)
            s_kv = persist.tile([128, MT], F32)
            ones = persist.tile([128, 1], BF16)
            warm = persist.tile([128, 512], BF16)
            nc.vector.memset(ones, 1.0)
            nc.vector.memset(warm, 0.001)

            # ---------------- phase A: col-major A GEMM (all bf16) ----------------
            wq_pre = {}
            wq8_pre = {}
            wkv8_pre = {}
            with tc.tile_pool(name="hst", bufs=1) as hstp, \
                 tc.tile_pool(name="wa", bufs=3) as wap, \
                 tc.tile_pool(name="sq", bufs=3) as sqp, \
                 tc.tile_pool(name="pe", bufs=1) as pep, \
                 tc.tile_pool(name="psA", bufs=3, space="PSUM") as psA, \
                 tc.tile_pool(name="sps", bufs=1, space="PSUM") as spsp:
                # PE warmup: dummy chained matmuls ramp the tensor engine's
                # pstate while the first input DMAs stream in
                ps_w = psA.tile([128, TPC], F32, tag="psA", name="psA_warm")
                for i in range(10):
                    nc.tensor.matmul(ps_w[:, 0:512], warm[:, 0:128], warm,
                                     start=(i == 0), stop=(i == 9),
                                     skip_group_check=True)

                hst = [hstp.tile([128, 4, TPC], BF16, name=f"hst{i}") for i in range(8)]
                # first-needed data split small and spread across queues so the
                # first matmul starts ~10us: cb0 k0-1 needs hst0a + wa0a only
                nc.sync.dma_start(out=hst[0][:, 0:2, :], in_=hsT_d[0][:, 0:2, :])
                nc.gpsimd.dma_start(out=hst[0][:, 2:4, :], in_=hsT_d[0][:, 2:4, :])
                for i in range(1, 8):
                    nc.scalar.dma_start(out=hst[i], in_=hsT_d[i])
                sps_q = spsp.tile([128, MT], F32)
                sps_kv = spsp.tile([128, MT], F32)
                sq_tiles = {}

                def stats_mms(pcb):
                    sq_t = sq_tiles.pop(pcb)
                    tgt, g0, g1 = (sps_q, 0, QKT - 1) if pcb < QKT else (sps_kv, QKT, 15)
                    for m in range(MT):
                        # start only on the bank's first write: start_tensor_calc
                        # clears has_written for the whole 2KB bank
                        nc.tensor.matmul(tgt[:, m:m + 1], sq_t[:, m * 128:(m + 1) * 128],
                                         ones, start=(pcb == g0 and m == 0),
                                         stop=(pcb == g1),
                                         skip_group_check=True)

                def a_mms(ps, wa_t, k0, k1):
                    # j outer: consecutive matmuls then use different stationary
                    # tiles, so LDWEIGHTS alternates weight buffers and hides
                    for j in range(2):
                        for k in range(k0, k1):
                            nc.tensor.matmul(ps[:, j * 512:(j + 1) * 512],
                                             wa_t[:, k, :],
                                             hst[k // 4][:, k % 4, j * 512:(j + 1) * 512],
                                             start=(k == 0), stop=(k == KT - 1),
                                             skip_group_check=True)

                def a_evict(cb, ps):
                    # at evicted as at*16: bf16 for B-q's bf16 k-tiles, fp8
                    # elsewhere; the 16*1024 product scale is folded into the
                    # rsqrt scale at B eviction
                    if cb < 2 * N8_BQ:
                        tgt = at8q[:, cb // 2, cb % 2, :]
                    elif cb < QKT:
                        tgt = at_sb[:, cb - 2 * N8_BQ, :]
                    else:
                        tgt = at8kv[:, (cb - QKT) // 2, (cb - QKT) % 2, :]
                    nc.scalar.activation(tgt, ps, AF.Copy, scale=SA)
                    sq_t = sqp.tile([128, TPC], BF16, tag="sq")
                    nc.scalar.activation(sq_t, ps, AF.Square)
                    sq_tiles[cb] = sq_t

                # cb0-2 interleaved in k-thirds: cuts the early hst consumption
                # rate so the initial hst DMA stream keeps up; cb0's first seg
                # further split so compute starts on hst0a+wa0a (0.5MB)
                was, pss = [], []
                for cb in range(3):
                    t = wap.tile([128, KT, 128], BF16, tag="wa", name=f"wa_s{cb}")
                    if cb == 0:
                        nc.sync.dma_start(out=t[:, 0:8, :], in_=waT_d[0][:, 0:8, :])
                        nc.gpsimd.dma_start(out=t[:, 8:, :], in_=waT_d[0][:, 8:, :])
                    else:
                        nc.sync.dma_start(out=t, in_=waT_d[cb])
                    was.append(t)
                    pss.append(psA.tile([128, TPC], F32, tag="psA", name=f"psA_s{cb}"))
                for seg, (k0, k1) in enumerate(((0, 2), (2, 8), (8, 11),
                                               (11, 22), (22, KT))):
                    for cb in range(3):
                        if seg < 2 and cb > 0:
                            continue        # cb1/cb2 start at seg2 (k0-10 whole)
                        if seg == 2 and cb > 0:
                            a_mms(pss[cb], was[cb], 0, 11)
                            continue
                        a_mms(pss[cb], was[cb], k0, k1)
                        if seg == 4:
                            a_evict(cb, pss[cb])
                stats_mms(0)
                stats_mms(1)

                for cb in range(3, ACB):
                    wa_t = wap.tile([128, KT, 128], BF16, tag="wa")
                    nc.sync.dma_start(out=wa_t, in_=waT_d[cb])
                    if cb == 14:
                        # prefetch the first B-phase weight tiles so phase B
                        # matmuls start without a DMA stall
                        for c in range(3):
                            t8 = wqp.tile([128, N8_BQ, 2, 256], F8, tag="wq8")
                            nc.sync.dma_start(out=t8, in_=qb8_d[c])
                            wq8_pre[c] = t8
                            t = wqp.tile([128, QKB, 256], BF16, tag="wq")
                            nc.sync.dma_start(out=t, in_=qbT_d[c])
                            wq_pre[c] = t
                        t8 = wkvp.tile([128, N8_BKV, 2, 256], F8, tag="wkv8")
                        nc.sync.dma_start(out=t8, in_=kvb8_d[0])
                        wkv8_pre[0] = t8
                    ps = psA.tile([128, TPC], F32, tag="psA")
                    a_mms(ps, wa_t, 0, KT)
                    stats_mms(cb - 1)       # slack: evict+square done ~13us ago
                    if cb < 16:
                        a_evict(cb, ps)
                    else:
                        pe_sb = pep.tile([128, TPC], BF16)
                        nc.scalar.activation(pe_sb[0:D_ROPE, :], ps[0:D_ROPE, :], AF.Copy)
                        nc.gpsimd.dma_start(out=ope_d[:, :], in_=pe_sb[0:D_ROPE, :])

                # per-token scales in [128 tok, m] layout; Sqrt scale 2^-28
                # bakes the 1/(SA*SWB) descale into the per-token rsqrt
                for sps, s_tok, dim in ((sps_q, s_q, float(RQ)), (sps_kv, s_kv, float(R_KV))):
                    nc.vector.tensor_scalar(out=s_tok, in0=sps, scalar1=1.0 / dim,
                                            scalar2=EPS, op0=ALU.mult, op1=ALU.add)
                    nc.vector.reciprocal(s_tok, s_tok)
                    nc.scalar.activation(s_tok, s_tok, AF.Sqrt, scale=SSQRT)

            # ---------------- phase B: token-major B GEMMs, q/kv interleaved ----------------
            with tc.tile_pool(name="stage", bufs=2) as stp, \
                 tc.tile_pool(name="ev", bufs=3) as evp, \
                 tc.tile_pool(name="psB", bufs=2, space="PSUM") as psB:
                stage = None
                for i in range(8):
                    for c in range(3 * i, 3 * i + 3):       # 3 q chunks (256 cols each)
                        if c in wq_pre:
                            wq_t = wq_pre.pop(c)
                            wq8_t = wq8_pre.pop(c)
                        else:
                            wq8_t = wqp.tile([128, N8_BQ, 2, 256], F8, tag="wq8")
                            nc.sync.dma_start(out=wq8_t, in_=qb8_d[c])
                            wq_t = wqp.tile([128, QKB, 256], BF16, tag="wq")
                            nc.sync.dma_start(out=wq_t, in_=qbT_d[c])
                        ps = psB.tile([128, MT, 256], F32, tag="psB")
                        for p in range(N8_BQ):
                            for m in range(MT):
                                # m-slices are 1KB: two share a 2KB psum bank, so
                                # only the even m's first matmul may set start
                                nc.tensor.matmul(ps[:, m, :],
                                                 at8q[:, p, :, m * 128:(m + 1) * 128],
                                                 wq8_t[:, p],
                                                 start=(p == 0 and m % 2 == 0),
                                                 stop=False, perf_mode=DR,
                                                 skip_group_check=True)
                        for k in range(QKB):
                            for m in range(MT):
                                nc.tensor.matmul(ps[:, m, :],
                                                 at_sb[:, k, m * 128:(m + 1) * 128],
                                                 wq_t[:, k, :],
                                                 start=False, stop=(k == QKB - 1),
                                                 skip_group_check=True)
                        if c % 3 == 0:
                            stage = stp.tile([128, MT, 768], BF16, tag="stage")
                        o0 = (c % 3) * 256
                        # scaled eviction: scalar does m0-1, vector does m2-7 in
                        # one broadcast multiply
                        for m in range(2):
                            nc.scalar.activation(stage[:, m, o0:o0 + 256], ps[:, m, :],
                                                 AF.Copy, scale=s_q[:, m:m + 1])
                        nc.vector.tensor_tensor(
                            out=stage[:, 2:8, o0:o0 + 256], in0=ps[:, 2:8, :],
                            in1=s_q[:, 2:8, None].broadcast_to([128, 6, 256]),
                            op=ALU.mult)
                        # heads complete progressively: c%3==0 -> h+0, ==1 -> h+1,
                        # ==2 -> h+2 and h+3 (keeps the output queue smooth)
                        h0 = (c // 3) * 4
                        for hh in ([0], [1], [2, 3])[c % 3]:
                            nc.gpsimd.dma_start(
                                out=oq_d[h0 + hh],
                                in_=stage[:, :, hh * D_Q:(hh + 1) * D_Q])
                    for c in range(4 * i, 4 * i + 4):       # 4 kv chunks (1 head each)
                        if c in wkv8_pre:
                            wkv8_t = wkv8_pre.pop(c)
                        else:
                            wkv8_t = wkvp.tile([128, N8_BKV, 2, 256], F8, tag="wkv8")
                            nc.sync.dma_start(out=wkv8_t, in_=kvb8_d[c])
                        ps = psB.tile([128, MT, 256], F32, tag="psB")
                        for p in range(N8_BKV):
                            for m in range(MT):
                                nc.tensor.matmul(ps[:, m, :],
                                                 at8kv[:, p, :, m * 128:(m + 1) * 128],
                                                 wkv8_t[:, p],
                                                 start=(p == 0 and m % 2 == 0),
                                                 stop=(p == N8_BKV - 1), perf_mode=DR,
                                                 skip_group_check=True)
                        ev = evp.tile([128, MT, 256], BF16, tag="ev")
                        msplit = 4 if c == KVCH - 1 else 2
                        for m in range(msplit):
                            nc.scalar.activation(ev[:, m, :], ps[:, m, :],
                                                 AF.Copy, scale=s_kv[:, m:m + 1])
                        nc.vector.tensor_tensor(
                            out=ev[:, msplit:8, :], in0=ps[:, msplit:8, :],
                            in1=s_kv[:, msplit:8, None].broadcast_to([128, 8 - msplit, 256]),
                            op=ALU.mult)
                        # spread the tail chunks' output DMAs across queues so
                        # the drain after the last matmul stays short
                        okn_eng = nc.scalar if c >= KVCH - 3 else nc.gpsimd
                        ov_eng = nc.sync if c >= KVCH - 2 else nc.scalar
                        okn_eng.dma_start(out=okn_d[c], in_=ev[:, :, 0:D_NOPE])
                        ov_eng.dma_start(out=ov_d[c], in_=ev[:, :, D_NOPE:256])

    nc.compile()
    _CACHE["nc"] = nc
    return nc


def _prep_inputs(hidden_states, q_a_w, kv_a_w, q_b_w, kv_b_w, q_a_ln_w, kv_a_ln_w):
    import ml_dtypes
    f32 = np.float32
    bf16 = ml_dtypes.bfloat16
    e4m3 = ml_dtypes.float8_e4m3
    perm = _perm64()

    q_a_w = np.asarray(q_a_w, dtype=f32)
    kv_a_w = np.asarray(kv_a_w, dtype=f32)
    wa = np.zeros((ACB * 128, DMODEL), dtype=f32)
    wa[:RQ] = q_a_w
    wa[RQ:RQ + R_KV] = kv_a_w[:R_KV]
    wa[RQ + R_KV:A_COLS] = kv_a_w[R_KV:][perm]             # de-interleave k_pe rows
    waT = np.ascontiguousarray(
        wa.reshape(ACB, 128, KT, 128).transpose(0, 3, 2, 1)).astype(bf16)

    qb = np.asarray(q_b_w, dtype=f32) * np.asarray(q_a_ln_w, dtype=f32)[None, :]
    qb = qb.reshape(H, D_Q, RQ).copy()
    qb[:, D_NOPE:, :] = qb[:, D_NOPE + perm, :]            # de-interleave q_pe rows
    qbt = (qb.reshape(QCH, 256, QKT, 128) * SWB).transpose(0, 3, 2, 1)
    qbT = np.ascontiguousarray(qbt[:, :, 2 * N8_BQ:, :]).astype(bf16)
    qb8 = np.clip(qbt[:, :, :2 * N8_BQ, :], -240, 240).astype(e4m3).reshape(
        QCH, 128, N8_BQ, 2, 256)
    qb8 = np.ascontiguousarray(qb8)

    kvb = np.asarray(kv_b_w, dtype=f32) * np.asarray(kv_a_ln_w, dtype=f32)[None, :]
    kvbt = (kvb.reshape(KVCH, 256, KVKT, 128) * SWB).transpose(0, 3, 2, 1)
    kvb8 = np.clip(kvbt, -240, 240).astype(e4m3).reshape(KVCH, 128, N8_BKV, 2, 256)
    kvb8 = np.ascontiguousarray(kvb8)

    hs = np.asarray(hidden_states, dtype=f32).reshape(NTOK, DMODEL)
    in_maps = []
    for c in range(NCORES):
        # hsT_d[i, p, kk, t] = hs[tok0+t, (i*4+kk)*128 + p]
        hsc = np.ascontiguousarray(
            hs[c * TPC:(c + 1) * TPC].reshape(TPC, KT, 128).transpose(2, 1, 0)
            .reshape(128, 8, 4, TPC).transpose(1, 0, 2, 3)).astype(bf16)
        in_maps.append({"hsT": hsc, "waT": waT, "qbT": qbT, "qb8": qb8,
                        "kvb8": kvb8})
    return in_maps


def kernel(hidden_states, q_a_w, q_b_w, kv_a_w, kv_b_w, q_a_ln_w, kv_a_ln_w,
           _trace=False):
    _ensure_env()
    from concourse.bass_utils import run_bass_kernel_spmd

    nc = _build()
    in_maps = _prep_inputs(hidden_states, q_a_w, kv_a_w, q_b_w, kv_b_w,
                           q_a_ln_w, kv_a_ln_w)
    res = run_bass_kernel_spmd(nc, in_maps, list(range(NCORES)), trace=_trace)

    out = np.empty((B, 3 * H, S, D_Q), dtype=np.float32)
    for c in range(NCORES):
        r = res.results[c]
        b = c // (S // TPC)
        sl = slice((c % (S // TPC)) * TPC, ((c % (S // TPC)) + 1) * TPC)
        # device layout [*, 128, MT, D]: token = m*128 + p
        out[b, :H, sl, :] = r["oq"].transpose(0, 2, 1, 3).reshape(
            H, TPC, D_Q).astype(np.float32)
        out[b, H:2 * H, sl, :D_NOPE] = r["okn"].transpose(0, 2, 1, 3).reshape(
            H, TPC, D_NOPE).astype(np.float32)
        out[b, H:2 * H, sl, D_NOPE:] = r["ope"].T.astype(np.float32)[None, :, :]
        out[b, 2 * H:, sl, :D_V] = r["ov"].transpose(0, 2, 1, 3).reshape(
            H, TPC, D_V).astype(np.float32)
    out[:, 2 * H:, :, D_V:] = 0.0
    if _trace:
        kernel.last_exec_time_ns = res.exec_time_ns
        kernel.last_results = res
    return out


# revision 13
# speedup vs baseline: 1.0589x; 1.0364x over previous
"""MLA QKV projection kernel v3 for Trainium2 (8 NeuronCores, Bass/Tile).

Strategy vs v2 baseline (643.3 us):
- Partial fp8: the first 4 k-tiles (of 12) of the B-q GEMM and ALL 4
  k-tiles of the B-kv GEMM run as fp8e4 DoubleRow matmuls (2 k-tiles
  contracted per call at bf16 per-call cost = 2x throughput). Saves
  ~99 us of PE time. Offline-calibrated rel_err 1.53e-2 < 2e-2 gate.
  Scales: at evicted as at*16 (both bf16 and e4m3), B weights hosted
  as w*1024; psum = (at@w)*2^14, descaled by folding 2^-14 into the
  rsqrt scale (Sqrt activation scale=2^-28).
- Outputs bf16 in partition-major layout [*, 128, MT, D]: contiguous
  2-3KB DMA descriptors per partition (v2's token-major rearrange made
  512B descriptors that left a 13 us descriptor-bound drain tail).
  Host upcasts/transposes for free.
- Head: first hst/wa tiles subtiled so the first matmul starts at
  ~10 us instead of 20.8; PE warmed up with dummy matmuls during the
  initial DMA wait (pstate ramp).
- RMSNorm sum-of-squares taken from the A psum (not the evicted bf16
  copy), squares via DVE, partition-reduction via per-(cb,m)
  ones-matmuls into token-partition layout [128, MT] as in v2.
"""

import sys
import types

import numpy as np

# ---- constants (hardcoded problem shape) ----
H = 32
D_NOPE = 128
D_ROPE = 64
D_Q = 192
D_V = 128
R_KV = 512
RQ = 1536
DMODEL = 4096
EPS = 1e-6
B, S = 2, 4096
NTOK = B * S            # 8192
NCORES = 8
TPC = NTOK // NCORES    # 1024 tokens per core
MT = TPC // 128         # 8 m-tiles
KT = DMODEL // 128      # 32 k-tiles for the A GEMM

A_COLS = RQ + R_KV + D_ROPE    # 2112
ACB = 17                       # a col-blocks of 128 (2176 padded; cb16 = k_pe+pad)
QKT = RQ // 128                # 12 contraction tiles for B-q
KVKT = R_KV // 128             # 4 for B-kv
QCH = 24                       # B-q chunks of 256 out cols (6144)
KVCH = 32                      # B-kv chunks of 256 out cols = 1 head (8192)

N8_BQ = 3                      # fp8 DoubleRow k-pairs in B-q (k-tiles 0..5)
N8_BKV = 2                     # fp8 DoubleRow k-pairs in B-kv (all 4 k-tiles)
QKB = QKT - 2 * N8_BQ          # bf16 k-tiles in B-q (8)
SA = 16.0                      # at fp8/bf16 eviction scale
SWB = 1024.0                   # B-weight host scale
SSQRT = 2.0 ** -28             # folds 1/(SA*SWB) into the rsqrt


def _ensure_env():
    for p in ("/opt/trn_rl_repo", "/root/.axon_site"):
        if p not in sys.path:
            sys.path.insert(0, p)
    if "antenv.axon_hooks" not in sys.modules:
        try:
            import antenv  # noqa: F401
            import antenv.axon_hooks  # noqa: F401
        except ImportError:
            mod = types.ModuleType("antenv.axon_hooks")
            mod._hook = None
            mod.set_axon_ntff_profile_hook = lambda h: setattr(mod, "_hook", h)
            mod.get_axon_ntff_profile_hook = lambda: mod._hook
            sys.modules["antenv.axon_hooks"] = mod
            try:
                import antenv
                antenv.axon_hooks = mod
            except ImportError:
                pass


def _perm64():
    # inverse view of x.reshape(32,2).swapaxes -> y[k] = x[2*(k%32) + k//32]
    return np.array([2 * (k % 32) + k // 32 for k in range(64)], dtype=np.int64)


_CACHE = {}


def _build():
    if "nc" in _CACHE:
        return _CACHE["nc"]
    _ensure_env()
    from concourse import bacc
    import concourse.mybir as mybir
    import concourse.tile as tile

    F32 = mybir.dt.float32
    BF16 = mybir.dt.bfloat16
    F8 = mybir.dt.float8e4
    AF = mybir.ActivationFunctionType
    ALU = mybir.AluOpType
    DR = mybir.MatmulPerfMode.DoubleRow

    nc = bacc.Bacc("TRN2", target_bir_lowering=False, debug=False)
    hsT_d = nc.dram_tensor("hsT", [8, 128, 4, TPC], BF16, kind="ExternalInput")
    waT_d = nc.dram_tensor("waT", [ACB, 128, KT, 128], BF16, kind="ExternalInput")
    qbT_d = nc.dram_tensor("qbT", [QCH, 128, QKB, 256], BF16, kind="ExternalInput")
    qb8_d = nc.dram_tensor("qb8", [QCH, 128, N8_BQ, 2, 256], F8, kind="ExternalInput")
    kvb8_d = nc.dram_tensor("kvb8", [KVCH, 128, N8_BKV, 2, 256], F8,
                            kind="ExternalInput")
    oq_d = nc.dram_tensor("oq", [H, 128, MT, D_Q], BF16, kind="ExternalOutput")
    okn_d = nc.dram_tensor("okn", [H, 128, MT, D_NOPE], BF16, kind="ExternalOutput")
    ov_d = nc.dram_tensor("ov", [H, 128, MT, D_V], BF16, kind="ExternalOutput")
    ope_d = nc.dram_tensor("ope", [D_ROPE, TPC], BF16, kind="ExternalOutput")

    with tile.TileContext(nc) as tc:
        with tc.tile_pool(name="persist", bufs=1) as persist, \
             tc.tile_pool(name="wq", bufs=3) as wqp, \
             tc.tile_pool(name="wkv", bufs=3) as wkvp:
            at_sb = persist.tile([128, QKB, TPC], BF16)        # bf16 at (q k4-11)
            at8q = persist.tile([128, N8_BQ, 2, TPC], F8)      # fp8 at (q k0-3)
            at8kv = persist.tile([128, N8_BKV, 2, TPC], F8)    # fp8 at (kv k0-3)
            s_q = persist.tile([128, MT], F32)
            s_kv = persist.tile([128, MT], F32)
            ones = persist.tile([128, 1], BF16)
            warm = persist.tile([128, 512], BF16)
            nc.vector.memset(ones, 1.0)
            nc.vector.memset(warm, 0.001)

            # ---------------- phase A: col-major A GEMM (all bf16) ----------------
            wq_pre = {}
            wq8_pre = {}
            wkv8_pre = {}
            with tc.tile_pool(name="hst", bufs=1) as hstp, \
                 tc.tile_pool(name="wa", bufs=3) as wap, \
                 tc.tile_pool(name="sq", bufs=3) as sqp, \
                 tc.tile_pool(name="pe", bufs=1) as pep, \
                 tc.tile_pool(name="psA", bufs=3, space="PSUM") as psA, \
                 tc.tile_pool(name="sps", bufs=1, space="PSUM") as spsp:
                # PE warmup: dummy chained matmuls ramp the tensor engine's
                # pstate while the first input DMAs stream in
                ps_w = psA.tile([128, TPC], F32, tag="psA", name="psA_warm")
                for i in range(10):
                    nc.tensor.matmul(ps_w[:, 0:512], warm[:, 0:128], warm,
                                     start=(i == 0), stop=(i == 9),
                                     skip_group_check=True)

                hst = [hstp.tile([128, 4, TPC], BF16, name=f"hst{i}") for i in range(8)]
                # first-needed data split small and spread across queues so the
                # first matmul starts ~10us: cb0 k0-1 needs hst0a + wa0a only
                nc.sync.dma_start(out=hst[0][:, 0:2, :], in_=hsT_d[0][:, 0:2, :])
                nc.gpsimd.dma_start(out=hst[0][:, 2:4, :], in_=hsT_d[0][:, 2:4, :])
                for i in range(1, 8):
                    nc.scalar.dma_start(out=hst[i], in_=hsT_d[i])
                sps_q = spsp.tile([128, MT], F32)
                sps_kv = spsp.tile([128, MT], F32)
                sq_tiles = {}

                def stats_mms(pcb):
                    sq_t = sq_tiles.pop(pcb)
                    tgt, g0, g1 = (sps_q, 0, QKT - 1) if pcb < QKT else (sps_kv, QKT, 15)
                    for m in range(MT):
                        # start only on the bank's first write: start_tensor_calc
                        # clears has_written for the whole 2KB bank
                        nc.tensor.matmul(tgt[:, m:m + 1], sq_t[:, m * 128:(m + 1) * 128],
                                         ones, start=(pcb == g0 and m == 0),
                                         stop=(pcb == g1),
                                         skip_group_check=True)

                def a_mms(ps, wa_t, k0, k1):
                    # j outer: consecutive matmuls then use different stationary
                    # tiles, so LDWEIGHTS alternates weight buffers and hides
                    for j in range(2):
                        for k in range(k0, k1):
                            nc.tensor.matmul(ps[:, j * 512:(j + 1) * 512],
                                             wa_t[:, k, :],
                                             hst[k // 4][:, k % 4, j * 512:(j + 1) * 512],
                                             start=(k == 0), stop=(k == KT - 1),
                                             skip_group_check=True)

                def a_evict(cb, ps):
                    # at evicted as at*16: bf16 for B-q's bf16 k-tiles, fp8
                    # elsewhere; the 16*1024 product scale is folded into the
                    # rsqrt scale at B eviction
                    if cb < 2 * N8_BQ:
                        tgt = at8q[:, cb // 2, cb % 2, :]
                    elif cb < QKT:
                        tgt = at_sb[:, cb - 2 * N8_BQ, :]
                    else:
                        tgt = at8kv[:, (cb - QKT) // 2, (cb - QKT) % 2, :]
                    nc.scalar.activation(tgt, ps, AF.Copy, scale=SA)
                    sq_t = sqp.tile([128, TPC], BF16, tag="sq")
                    nc.scalar.activation(sq_t, ps, AF.Square)
                    sq_tiles[cb] = sq_t

                # cb0-2 interleaved in k-thirds: cuts the early hst consumption
                # rate so the initial hst DMA stream keeps up; cb0's first seg
                # further split so compute starts on hst0a+wa0a (0.5MB)
                was, pss = [], []
                for cb in range(3):
                    t = wap.tile([128, KT, 128], BF16, tag="wa", name=f"wa_s{cb}")
                    if cb == 0:
                        nc.sync.dma_start(out=t[:, 0:8, :], in_=waT_d[0][:, 0:8, :])
                        nc.gpsimd.dma_start(out=t[:, 8:, :], in_=waT_d[0][:, 8:, :])
                    else:
                        nc.sync.dma_start(out=t, in_=waT_d[cb])
                    was.append(t)
                    pss.append(psA.tile([128, TPC], F32, tag="psA", name=f"psA_s{cb}"))
                for seg, (k0, k1) in enumerate(((0, 2), (2, 8), (8, 11),
                                               (11, 22), (22, KT))):
                    for cb in range(3):
                        if seg < 2 and cb > 0:
                            continue        # cb1/cb2 start at seg2 (k0-10 whole)
                        if seg == 2 and cb > 0:
                            a_mms(pss[cb], was[cb], 0, 11)
                            continue
                        a_mms(pss[cb], was[cb], k0, k1)
                        if seg == 4:
                            a_evict(cb, pss[cb])
                stats_mms(0)
                stats_mms(1)

                for cb in range(3, ACB):
                    wa_t = wap.tile([128, KT, 128], BF16, tag="wa")
                    nc.sync.dma_start(out=wa_t, in_=waT_d[cb])
                    if cb == 14:
                        # prefetch the first B-phase weight tiles so phase B
                        # matmuls start without a DMA stall
                        for c in range(3):
                            t8 = wqp.tile([128, N8_BQ, 2, 256], F8, tag="wq8")
                            nc.sync.dma_start(out=t8, in_=qb8_d[c])
                            wq8_pre[c] = t8
                            t = wqp.tile([128, QKB, 256], BF16, tag="wq")
                            nc.sync.dma_start(out=t, in_=qbT_d[c])
                            wq_pre[c] = t
                        t8 = wkvp.tile([128, N8_BKV, 2, 256], F8, tag="wkv8")
                        nc.sync.dma_start(out=t8, in_=kvb8_d[0])
                        wkv8_pre[0] = t8
                    ps = psA.tile([128, TPC], F32, tag="psA")
                    a_mms(ps, wa_t, 0, KT)
                    stats_mms(cb - 1)       # slack: evict+square done ~13us ago
                    if cb < 16:
                        a_evict(cb, ps)
                    else:
                        pe_sb = pep.tile([128, TPC], BF16)
                        nc.scalar.activation(pe_sb[0:D_ROPE, :], ps[0:D_ROPE, :], AF.Copy)
                        nc.gpsimd.dma_start(out=ope_d[:, :], in_=pe_sb[0:D_ROPE, :])

                # per-token scales in [128 tok, m] layout; Sqrt scale 2^-28
                # bakes the 1/(SA*SWB) descale into the per-token rsqrt
                for sps, s_tok, dim in ((sps_q, s_q, float(RQ)), (sps_kv, s_kv, float(R_KV))):
                    nc.vector.tensor_scalar(out=s_tok, in0=sps, scalar1=1.0 / dim,
                                            scalar2=EPS, op0=ALU.mult, op1=ALU.add)
                    nc.vector.reciprocal(s_tok, s_tok)
                    nc.scalar.activation(s_tok, s_tok, AF.Sqrt, scale=SSQRT)

            # ---------------- phase B: token-major B GEMMs, q/kv interleaved ----------------
            with tc.tile_pool(name="stage", bufs=2) as stp, \
                 tc.tile_pool(name="ev", bufs=3) as evp, \
                 tc.tile_pool(name="psB", bufs=2, space="PSUM") as psB:
                stage = None
                for i in range(8):
                    for c in range(3 * i, 3 * i + 3):       # 3 q chunks (256 cols each)
                        if c in wq_pre:
                            wq_t = wq_pre.pop(c)
                            wq8_t = wq8_pre.pop(c)
                        else:
                            wq8_t = wqp.tile([128, N8_BQ, 2, 256], F8, tag="wq8")
                            nc.sync.dma_start(out=wq8_t, in_=qb8_d[c])
                            wq_t = wqp.tile([128, QKB, 256], BF16, tag="wq")
                            nc.sync.dma_start(out=wq_t, in_=qbT_d[c])
                        ps = psB.tile([128, MT, 256], F32, tag="psB")
                        for p in range(N8_BQ):
                            for m in range(MT):
                                # m-slices are 1KB: two share a 2KB psum bank, so
                                # only the even m's first matmul may set start
                                nc.tensor.matmul(ps[:, m, :],
                                                 at8q[:, p, :, m * 128:(m + 1) * 128],
                                                 wq8_t[:, p],
                                                 start=(p == 0 and m % 2 == 0),
                                                 stop=False, perf_mode=DR,
                                                 skip_group_check=True)
                        for k in range(QKB):
                            for m in range(MT):
                                nc.tensor.matmul(ps[:, m, :],
                                                 at_sb[:, k, m * 128:(m + 1) * 128],
                                                 wq_t[:, k, :],
                                                 start=False, stop=(k == QKB - 1),
                                                 skip_group_check=True)
                        if c % 3 == 0:
                            stage = stp.tile([128, MT, 768], BF16, tag="stage")
                        o0 = (c % 3) * 256
                        # scaled eviction: scalar does m0-1, vector does m2-7 in
                        # one broadcast multiply
                        for m in range(2):
                            nc.scalar.activation(stage[:, m, o0:o0 + 256], ps[:, m, :],
                                                 AF.Copy, scale=s_q[:, m:m + 1])
                        nc.vector.tensor_tensor(
                            out=stage[:, 2:8, o0:o0 + 256], in0=ps[:, 2:8, :],
                            in1=s_q[:, 2:8, None].broadcast_to([128, 6, 256]),
                            op=ALU.mult)
                        # heads complete progressively: c%3==0 -> h+0, ==1 -> h+1,
                        # ==2 -> h+2 and h+3 (keeps the output queue smooth)
                        h0 = (c // 3) * 4
                        for hh in ([0], [1], [2, 3])[c % 3]:
                            nc.gpsimd.dma_start(
                                out=oq_d[h0 + hh],
                                in_=stage[:, :, hh * D_Q:(hh + 1) * D_Q])
                    for c in range(4 * i, 4 * i + 4):       # 4 kv chunks (1 head each)
                        if c in wkv8_pre:
                            wkv8_t = wkv8_pre.pop(c)
                        else:
                            wkv8_t = wkvp.tile([128, N8_BKV, 2, 256], F8, tag="wkv8")
                            nc.sync.dma_start(out=wkv8_t, in_=kvb8_d[c])
                        ps = psB.tile([128, MT, 256], F32, tag="psB")
                        for p in range(N8_BKV):
                            for m in range(MT):
                                nc.tensor.matmul(ps[:, m, :],
                                                 at8kv[:, p, :, m * 128:(m + 1) * 128],
                                                 wkv8_t[:, p],
                                                 start=(p == 0 and m % 2 == 0),
                                                 stop=(p == N8_BKV - 1), perf_mode=DR,
                                                 skip_group_check=True)
                        ev = evp.tile([128, MT, 256], BF16, tag="ev")
                        msplit = 4 if c == KVCH - 1 else 2
                        for m in range(msplit):
                            nc.scalar.activation(ev[:, m, :], ps[:, m, :],
                                                 AF.Copy, scale=s_kv[:, m:m + 1])
                        nc.vector.tensor_tensor(
                            out=ev[:, msplit:8, :], in0=ps[:, msplit:8, :],
                            in1=s_kv[:, msplit:8, None].broadcast_to([128, 8 - msplit, 256]),
                            op=ALU.mult)
                        # spread the tail chunks' output DMAs across queues so
                        # the drain after the last matmul stays short
                        okn_eng = nc.scalar if c >= KVCH - 3 else nc.gpsimd
                        ov_eng = nc.sync if c >= KVCH - 2 else nc.scalar
                        okn_eng.dma_start(out=okn_d[c], in_=ev[:, :, 0:D_NOPE])
                        ov_eng.dma_start(out=ov_d[c], in_=ev[:, :, D_NOPE:256])

    nc.compile()
    _CACHE["nc"] = nc
    return nc


def _prep_inputs(hidden_states, q_a_w, kv_a_w, q_b_w, kv_b_w, q_a_ln_w, kv_a_ln_w):
    import ml_dtypes
    f32 = np.float32
    bf16 = ml_dtypes.bfloat16
    e4m3 = ml_dtypes.float8_e4m3
    perm = _perm64()

    q_a_w = np.asarray(q_a_w, dtype=f32)
    kv_a_w = np.asarray(kv_a_w, dtype=f32)
    wa = np.zeros((ACB * 128, DMODEL), dtype=f32)
    wa[:RQ] = q_a_w
    wa[RQ:RQ + R_KV] = kv_a_w[:R_KV]
    wa[RQ + R_KV:A_COLS] = kv_a_w[R_KV:][perm]             # de-interleave k_pe rows
    waT = np.ascontiguousarray(
        wa.reshape(ACB, 128, KT, 128).transpose(0, 3, 2, 1)).astype(bf16)

    qb = np.asarray(q_b_w, dtype=f32) * np.asarray(q_a_ln_w, dtype=f32)[None, :]
    qb = qb.reshape(H, D_Q, RQ).copy()
    qb[:, D_NOPE:, :] = qb[:, D_NOPE + perm, :]            # de-interleave q_pe rows
    qbt = (qb.reshape(QCH, 256, QKT, 128) * SWB).transpose(0, 3, 2, 1)
    qbT = np.ascontiguousarray(qbt[:, :, 2 * N8_BQ:, :]).astype(bf16)
    qb8 = np.clip(qbt[:, :, :2 * N8_BQ, :], -240, 240).astype(e4m3).reshape(
        QCH, 128, N8_BQ, 2, 256)
    qb8 = np.ascontiguousarray(qb8)

    kvb = np.asarray(kv_b_w, dtype=f32) * np.asarray(kv_a_ln_w, dtype=f32)[None, :]
    kvbt = (kvb.reshape(KVCH, 256, KVKT, 128) * SWB).transpose(0, 3, 2, 1)
    kvb8 = np.clip(kvbt, -240, 240).astype(e4m3).reshape(KVCH, 128, N8_BKV, 2, 256)
    kvb8 = np.ascontiguousarray(kvb8)

    hs = np.asarray(hidden_states, dtype=f32).reshape(NTOK, DMODEL)
    in_maps = []
    for c in range(NCORES):
        # hsT_d[i, p, kk, t] = hs[tok0+t, (i*4+kk)*128 + p]
        hsc = np.ascontiguousarray(
            hs[c * TPC:(c + 1) * TPC].reshape(TPC, KT, 128).transpose(2, 1, 0)
            .reshape(128, 8, 4, TPC).transpose(1, 0, 2, 3)).astype(bf16)
        in_maps.append({"hsT": hsc, "waT": waT, "qbT": qbT, "qb8": qb8,
                        "kvb8": kvb8})
    return in_maps


def kernel(hidden_states, q_a_w, q_b_w, kv_a_w, kv_b_w, q_a_ln_w, kv_a_ln_w,
           _trace=False):
    _ensure_env()
    from concourse.bass_utils import run_bass_kernel_spmd

    nc = _build()
    in_maps = _prep_inputs(hidden_states, q_a_w, kv_a_w, q_b_w, kv_b_w,
                           q_a_ln_w, kv_a_ln_w)
    res = run_bass_kernel_spmd(nc, in_maps, list(range(NCORES)), trace=_trace)

    out = np.empty((B, 3 * H, S, D_Q), dtype=np.float32)
    for c in range(NCORES):
        r = res.results[c]
        b = c // (S // TPC)
        sl = slice((c % (S // TPC)) * TPC, ((c % (S // TPC)) + 1) * TPC)
        # device layout [*, 128, MT, D]: token = m*128 + p
        out[b, :H, sl, :] = r["oq"].transpose(0, 2, 1, 3).reshape(
            H, TPC, D_Q).astype(np.float32)
        out[b, H:2 * H, sl, :D_NOPE] = r["okn"].transpose(0, 2, 1, 3).reshape(
            H, TPC, D_NOPE).astype(np.float32)
        out[b, H:2 * H, sl, D_NOPE:] = r["ope"].T.astype(np.float32)[None, :, :]
        out[b, 2 * H:, sl, :D_V] = r["ov"].transpose(0, 2, 1, 3).reshape(
            H, TPC, D_V).astype(np.float32)
    out[:, 2 * H:, :, D_V:] = 0.0
    if _trace:
        kernel.last_exec_time_ns = res.exec_time_ns
        kernel.last_results = res
    return out
